# revision 32
# baseline (speedup 1.0000x reference)
"""Trainium2 Bass kernel for a custom LSTM cell.

reference:
    z = concat([h_tm1, inputs], -1) @ kernel      # [B, 4U]
    i, f, g, o = split(z, 4, -1)
    c = sigmoid(f) * c_tm1 + sigmoid(i) * tanh(g)
    h = sigmoid(o) * tanh(c)
    returns (h, c)

Sharding over 8 NeuronCores: 2-way over batch x 4-way over units
(each gate's block co-located per core).  Matmul inputs are quantized
to fp16 host-side (h rel-err ~2e-3, well under the 2e-2 budget) which
halves HBM traffic vs fp32r at the same PE rate.

Per core the activations and weights are packed into ONE dram tensor
`awk` [K, 2048] = [wk_lo 512 | at batch 1024 | wk_hi 512] per k-row,
so each k-subtile's phase-1 data arrives as a single DMA (descriptor
generation at ~650ns/DMA would otherwise outpace the transfers), and
the first chunk [wk_lo(k0) | first two batch tiles] is contiguous.

Schedule (per core):
  warmup: small dummy matmuls keep the PE busy from ~0.5us so the
          p-state clock is at 2.4GHz when real data lands (~3.9us);
          a dummy Sigmoid preloads the Act table (serves Tanh too).
  phase 1: k-outer round-robin over 8 PSUM groups (batch sub-tiles x
           i|f columns) consuming awk chunks in arrival order; each
           group closes with Sigmoid -> sig_if, then f*c_tm1 on DVE.
  phase 2: m7's g-group runs FIRST so its tanh/ig/cn/tanh(c) chain and
           c_out DMA complete ~17us before the end; then m0..m6
           (g then o per m, separate PSUM tiles to avoid false WAR
           deps), with m7's o-gate last, split into two 128-col groups
           so the final sigmoid->mul->DMA tail is half width.
All input DMAs stream on the Sync queue in consumption order; wk_hi
and c_tm1 ride at the end of the stream (first needed by phase 2).
Outputs leave fp16 on the Sync queue (HWDGE descriptor generation,
keeping Pool's slow SWDGE off the tail; the host casts back to f32).
"""

import sys

sys.path.insert(0, "/opt/trn_rl_repo")

import numpy as np

BATCH, INPUT_DIM, UNITS = 2048, 512, 1024
K = UNITS + INPUT_DIM  # contraction dim, 1536
R, C = 2, 4  # batch halves x unit quarters
BR = BATCH // R  # 1024 batch rows per core
UC = UNITS // C  # 256 units per core
KS = K // 128  # 12 k-subtiles
M = BR // 128  # 8 batch sub-chunks per core
AW = BR + 1024  # awk row width: wk_lo 512 | at 1024 | wk_hi 512
LO = 0  # wk_lo column offset in awk
AT = 512  # at column offset in awk
HI = AT + BR  # wk_hi column offset in awk

_CACHE = {}


def _build_nc():
    import concourse.tile as tile
    from concourse import bacc, mybir

    f32 = mybir.dt.float32
    f16 = mybir.dt.float16
    Sig = mybir.ActivationFunctionType.Sigmoid
    Tanh = mybir.ActivationFunctionType.Tanh

    nc = bacc.Bacc("TRN2")
    awk_in = nc.declare_dram_parameter("awk", [K, AW], f16, isOutput=False)
    ct_in = nc.declare_dram_parameter("ct", [BR, UC], f32, isOutput=False)
    # fp16 outputs: adds ~5e-4 rel rounding (budget is 2e-2), halves the
    # output DMA bytes — the final h transfer is on the kernel tail
    h_out = nc.declare_dram_parameter("h_out", [BR, UC], f16, isOutput=True)
    c_out = nc.declare_dram_parameter("c_out", [BR, UC], f16, isOutput=True)

    with tile.TileContext(nc) as tc:
        with (
            tc.tile_pool(name="data", bufs=1) as data,
            tc.tile_pool(name="work", bufs=3) as work,
            tc.tile_pool(name="psum", bufs=8, space="PSUM") as psum,
        ):
            awk = data.tile([128, KS, AW], f16)
            ct = data.tile([128, M, UC], f32)
            sig_if = data.tile([128, M, 512], f32)
            fc_all = data.tile([128, M, UC], f32)
            dum_w = data.tile([128, 128], f16)
            dum_a = data.tile([128, 8], f32)
            dum_o = data.tile([128, 8], f32)

            awk_r = awk_in[:].rearrange("(ko p) n -> p ko n", p=128)
            ct_r = ct_in[:].rearrange("(m p) u -> p m u", p=128)

            # warmup PSUM tile allocated first: the warmups then never touch
            # a real accumulation bank (unordered writes there corrupt the
            # i/f sub-groups below).  plo7 reuses this slot ~2us after the
            # last warmup retires.
            pwu = psum.tile([128, 512], f32, tag="ps", name="pwu")
            plo = [
                psum.tile([128, 512], f32, tag="ps", name=f"plo{m}") for m in range(M)
            ]

            # PE p-state warmup: 128-wide dummy matmuls from ~0.5us until
            # real data lands, so real matmuls start at full clock.
            nc.vector.memset(dum_w[:], 0.0)
            for _ in range(22):
                nc.tensor.matmul(
                    pwu[:, 0:128], dum_w[:], dum_w[:], start=True, stop=True,
                    skip_group_check=True,
                )
            # Act table preload (Sigmoid's table also serves Tanh).  All
            # memsets ride the DVE queue: keeping Pool fully unused drops
            # it from the end-of-kernel barrier.
            nc.vector.memset(dum_a[:], 0.0)
            nc.scalar.activation(dum_o[:], dum_a[:], Sig)

            # input stream, Sync queue, consumption order.  First chunk =
            # wk_lo(k0) + the first two batch tiles (contiguous thanks to
            # the wk_lo|at|wk_hi layout) so the PE starts ~550ns sooner;
            # wk_hi ships in bulk at the end (first needed by phase 2),
            # then c_tm1.
            nc.sync.dma_start(awk[:, 0:1, 0 : AT + 256], awk_r[:, 0:1, 0 : AT + 256])
            nc.sync.dma_start(awk[:, 0:1, AT + 256 : HI], awk_r[:, 0:1, AT + 256 : HI])
            for k in range(1, KS):
                nc.sync.dma_start(awk[:, k : k + 1, 0:HI], awk_r[:, k : k + 1, 0:HI])
            for j in range(3):
                ks4 = slice(4 * j, 4 * j + 4)
                nc.sync.dma_start(awk[:, ks4, HI:AW], awk_r[:, ks4, HI:AW])
            for j in range(2):
                ms4 = slice(4 * j, 4 * j + 4)
                nc.sync.dma_start(ct[:, ms4, :], ct_r[:, ms4, :])

            # phase 1: all m, i|f columns, k-outer round-robin.  (Keep each
            # bank's accumulation as ONE 512-wide group: interleaving two
            # k-streamed sub-groups in a single bank silently corrupts on
            # the hardware/compiler path.)
            for k in range(KS):
                for m in range(M):
                    nc.tensor.matmul(
                        plo[m][:],
                        awk[:, k, AT + m * 128 : AT + (m + 1) * 128],
                        awk[:, k, LO : LO + 512],
                        start=(k == 0),
                        stop=(k == KS - 1),
                    )
            for m in range(M):
                nc.scalar.activation(sig_if[:, m, :], plo[m][:], Sig)
            for m in range(M):
                nc.vector.tensor_mul(
                    fc_all[:, m, :], sig_if[:, m, UC : 2 * UC], ct[:, m, :]
                )

            def g_group(m, pt):
                ms = slice(m * 128, (m + 1) * 128)
                for k in range(KS):
                    nc.tensor.matmul(
                        pt[:, 0:UC],
                        awk[:, k, AT + ms.start : AT + ms.stop],
                        awk[:, k, HI : HI + UC],
                        start=(k == 0),
                        stop=(k == KS - 1),
                    )

            def o_group(m, pt, osl):
                ms = slice(m * 128, (m + 1) * 128)
                for k in range(KS):
                    nc.tensor.matmul(
                        pt[:, osl],
                        awk[:, k, AT + ms.start : AT + ms.stop],
                        awk[:, k, HI + UC + osl.start : HI + UC + osl.stop],
                        start=(k == 0),
                        stop=(k == KS - 1),
                    )

            # phase 2a: m7's g-group first; its tanh/ig/cn/c-out/tanh(c)
            # chain completes early, off the kernel tail.
            m7 = M - 1
            ms7 = slice(m7 * 128, (m7 + 1) * 128)
            phg7 = psum.tile([128, 512], f32, tag="ps", name="phg7")
            g_group(m7, phg7)
            tg7 = work.tile([128, UC], f32, tag="tg")
            nc.scalar.activation(tg7[:], phg7[:, 0:UC], Tanh)
            ig7 = work.tile([128, UC], f32, tag="ig")
            nc.vector.tensor_mul(ig7[:], sig_if[:, m7, 0:UC], tg7[:])
            cn7 = work.tile([128, UC], f16, tag="cn")
            nc.vector.tensor_add(cn7[:], fc_all[:, m7, :], ig7[:])
            nc.sync.dma_start(c_out[ms7, :], cn7[:])
            # th7 lives until the kernel tail — keep it out of the rotating
            # work pool (a later th alloc reusing its buffer would deadlock
            # the in-order Act queue against hn7's sigmoid).
            th7 = data.tile([128, UC], f16)
            nc.scalar.activation(th7[:], cn7[:], Tanh)

            # phase 2b: m0..m6, g then o per m, pipelined epilogues
            for m in range(M - 1):
                ms = slice(m * 128, (m + 1) * 128)
                phg = psum.tile([128, 512], f32, tag="ps", name=f"phg{m}")
                g_group(m, phg)
                tg = work.tile([128, UC], f32, tag="tg")
                nc.scalar.activation(tg[:], phg[:, 0:UC], Tanh)
                pho = psum.tile([128, 512], f32, tag="ps", name=f"pho{m}")
                o_group(m, pho, slice(0, UC))
                so = work.tile([128, UC], f16, tag="so")
                nc.scalar.activation(so[:], pho[:, 0:UC], Sig)
                ig = work.tile([128, UC], f32, tag="ig")
                nc.vector.tensor_mul(ig[:], sig_if[:, m, 0:UC], tg[:])
                cn = work.tile([128, UC], f16, tag="cn")
                nc.vector.tensor_add(cn[:], fc_all[:, m, :], ig[:])
                nc.sync.dma_start(c_out[ms, :], cn[:])
                th = work.tile([128, UC], f16, tag="th")
                nc.scalar.activation(th[:], cn[:], Tanh)
                hn = work.tile([128, UC], f16, tag="hn")
                nc.vector.tensor_mul(hn[:], so[:], th[:])
                nc.sync.dma_start(h_out[ms, :], hn[:])

            # phase 2c: m7's o-gate last, two 128-col groups in separate
            # PSUM tiles so each sigmoid fires as its half closes.
            hn7 = work.tile([128, UC], f16, tag="hn")
            for half in range(2):
                osl = slice(128 * half, 128 * (half + 1))
                pho = psum.tile([128, 512], f32, tag="ps", name=f"pho7{half}")
                o_group(m7, pho, osl)
                so = work.tile([128, UC], f16, tag="so")
                nc.scalar.activation(so[:, osl], pho[:, osl], Sig)
                nc.vector.tensor_mul(hn7[:, osl], so[:, osl], th7[:, osl])
            # single h7 DMA: two would serialize 625ns descriptor gens on
            # the shared HWDGE right at the kernel tail
            nc.sync.dma_start(h_out[ms7, :], hn7[:])

    nc.compile()
    return nc


def get_nc():
    if "nc" not in _CACHE:
        _CACHE["nc"] = _build_nc()
    return _CACHE["nc"]


def make_in_maps(inputs, h_tm1, c_tm1, kernel):
    x = np.asarray(inputs, dtype=np.float32)
    h = np.asarray(h_tm1, dtype=np.float32)
    c = np.ascontiguousarray(np.asarray(c_tm1, dtype=np.float32))
    w16 = np.asarray(kernel, dtype=np.float32).astype(np.float16)
    at_full = np.concatenate([h, x], axis=1).T.astype(np.float16)  # [K, B]
    in_maps = []
    for core in range(R * C):
        r, ci = divmod(core, C)
        at_np = at_full[:, r * BR : (r + 1) * BR]
        gates = [
            w16[:, g * UNITS + ci * UC : g * UNITS + (ci + 1) * UC] for g in range(4)
        ]
        awk_np = np.ascontiguousarray(
            np.concatenate(gates[0:2] + [at_np] + gates[2:4], axis=1)
        )  # [K, wk_lo 512 | at 1024 | wk_hi 512] fp16
        ct_np = np.ascontiguousarray(c[r * BR : (r + 1) * BR, ci * UC : (ci + 1) * UC])
        in_maps.append({"awk": awk_np, "ct": ct_np})
    return in_maps


def assemble(results):
    h_new = np.empty((BATCH, UNITS), dtype=np.float32)
    c_new = np.empty((BATCH, UNITS), dtype=np.float32)
    for core in range(R * C):
        r, ci = divmod(core, C)
        h_new[r * BR : (r + 1) * BR, ci * UC : (ci + 1) * UC] = results[core][
            "h_out"
        ].astype(np.float32)
        c_new[r * BR : (r + 1) * BR, ci * UC : (ci + 1) * UC] = results[core][
            "c_out"
        ].astype(np.float32)
    return h_new, c_new


def kernel(inputs, h_tm1, c_tm1, kernel):
    from concourse.bass_utils import run_bass_kernel_spmd

    nc = get_nc()
    in_maps = make_in_maps(inputs, h_tm1, c_tm1, kernel)
    res = run_bass_kernel_spmd(nc, in_maps, list(range(R * C)), trace=False)
    return assemble(res.results)
